# revision 27
# baseline (speedup 1.0000x reference)
"""BinaryConv (BN + sign-binarize + 3x3 binary conv) on 8 Trainium2 NeuronCores.

Strategy (data-parallel over batch, per sharding hint):
  - Each of the 8 cores gets 4 of the 32 images; weights/gamma/beta replicated.
  - Weights are host-prepped (sign + transpose to DoubleRow [ci][tap*o][half][o]
    fp8 layout) since they are tiny replicated constants; the device loads the
    packed 590KB tensor directly.
  - A tiny dummy AllGather is triggered first so the one-time collective
    rendezvous/arming cost overlaps the x load + local-stats phase instead of
    sitting on the critical path of the real stats AllGather.
  - Per-core BN partial stats (mean, mean-square per channel) via bn_stats,
    AllGathered across cores (tiny 2KB payload) and tree-reduced locally on
    DVE: sync-BN exact.
  - Binarize via ScalarE Sign(gamma*x + (beta*std - mean*gamma)) — equivalent
    to the reference BN+sign since std>0 — into a zero-padded per-image
    layout (34-wide rows, both ci-halves stacked) in fp8 e4m3.
  - 3x3 conv = 9 shifted DoubleRow fp8 matmuls (contraction 256 in one pass)
    accumulated in PSUM. +/-1 inputs with fp32 PSUM accumulation are exact
    (integer sums), so the conv matches the fp32 reference bit-for-bit.
  - A long chain of PE warm-up matmuls spans the stats/collective window so
    the HAM clock gate is fully ramped when the conv burst starts.
"""

import numpy as np

import concourse.tile as tile
from concourse import bacc, mybir
from concourse.bass_utils import run_bass_kernel_spmd
from concourse.masks import make_identity

F32 = mybir.dt.float32
BF16 = mybir.dt.bfloat16
FP8 = mybir.dt.float8e4

N_CORES = 8
N = 32            # full batch
NLOC = N // N_CORES  # images per core
C = 256           # channels (in == out)
HW = 32           # spatial
CB = C // 128     # ci partition blocks
OB = C // 128     # o partition blocks
EPS = 1e-5

PADW = HW + 2     # padded row width
IMG_PAD = 1160    # per-image padded buffer (>= 34*34 + 2 margin, mult of 8)
# output row-chunks (r0, r1): each chunk's matmul free dim = (r1-r0)*34 <= 512
CHUNKS = [(0, 11), (11, 22), (22, 32)]
TAPS = [(dy, dx) for dy in range(3) for dx in range(3)]
N_PRE_WARM = 52   # PE ramp matmuls spanning the stats/collective window


def _build_body(ctx, nc, tc, x_d, g_d, be_d, w_d, y_d, cc_in, cc_out):
    # ---------------- pools ----------------
    const = ctx.enter_context(tc.tile_pool(name="const", bufs=1))
    xin_p = ctx.enter_context(tc.tile_pool(name="xin", bufs=1))
    wpool = ctx.enter_context(tc.tile_pool(name="wpool", bufs=1))
    apool = ctx.enter_context(tc.tile_pool(name="apool", bufs=1))
    stat_p = ctx.enter_context(tc.tile_pool(name="stat", bufs=1))
    out_p = ctx.enter_context(tc.tile_pool(name="outp", bufs=1))
    ps_tr = ctx.enter_context(tc.tile_pool(name="pstr", bufs=2, space="PSUM"))
    ps_acc = ctx.enter_context(tc.tile_pool(name="psacc", bufs=1, space="PSUM"))

    # ---------------- load x (stats-critical) ----------------
    # Alternate the two HWDGE issue engines (Sync, Scalar) to halve the
    # descriptor-generation serialization on the input path.
    xin = []
    for b in range(CB):
        xb = xin_p.tile([128, NLOC, HW, HW], F32, name=f"xin{b}", tag=f"xin{b}")
        for i in range(NLOC):
            eng = nc.sync if (b * NLOC + i) % 2 == 0 else nc.scalar
            eng.dma_start(
                out=xb[:, i], in_=x_d[i, 128 * b : 128 * (b + 1), :, :]
            )
        xin.append(xb)

    # prepacked binarized weights, DoubleRow layout (host-prepared):
    # wT[ci_local, tap*OB+o, ci_half, o_local]  (fp8)
    wT = wpool.tile([128, len(TAPS) * OB, CB, 128], FP8, name="wT")
    nc.scalar.dma_start(out=wT[:], in_=w_d[:, :, :, :])

    # gamma/beta: tiny, issue right after the x/w descriptors
    gam = stat_p.tile([128, CB], F32, name="gam")
    bet = stat_p.tile([128, CB], F32, name="bet")
    for b in range(CB):
        nc.sync.dma_start(out=gam[:, b : b + 1], in_=g_d[128 * b : 128 * (b + 1), :])
        nc.sync.dma_start(out=bet[:, b : b + 1], in_=be_d[128 * b : 128 * (b + 1), :])

    # ---------------- scalar act-table preloads (Sign + Sqrt) ---------------
    eps_t = const.tile([128, 1], F32, name="eps_t")
    nc.vector.memset(eps_t[:], EPS)
    scrap = const.tile([128, 1], F32, name="scrap")
    nc.scalar.activation(
        out=scrap[:], in_=eps_t[:], func=mybir.ActivationFunctionType.Sign
    )
    nc.scalar.activation(
        out=scrap[:],
        in_=eps_t[:],
        func=mybir.ActivationFunctionType.Sqrt,
        bias=eps_t[:],
        scale=1.0,
    )

    # ---------------- PE warm-up chain through the stats window -------------
    # fp32 matmuls (4 cyc/row) give long-running filler with few
    # instructions: ~52 x ~850ns spans ~8..52us, keeping the PE clock
    # ramped until just before the conv burst.
    ident = const.tile([128, 128], BF16, name="ident")
    make_identity(nc, ident[:])
    jw = const.tile([128, 256], BF16, name="jw")
    nc.vector.memset(jw[:], 1.0)
    jwf = const.tile([128, 512], F32, name="jwf")
    nc.vector.memset(jwf[:], 1.0)
    for k in range(N_PRE_WARM):
        pw = ps_tr.tile([128, 512], F32, name="pw", tag="ptr", bufs=2)
        nc.tensor.matmul(pw[:], jwf[:, 0:128], jwf[:], start=True, stop=True)

    # ---------------- zero only the padding of the activation buffers ------
    # (interior is fully overwritten by binarize; tiny strided memsets keep
    # both DVE and the collective-trigger path free)
    apad = [None] * NLOC
    for i in range(NLOC):
        ap = apool.tile([128, CB, IMG_PAD], FP8, name=f"apad{i}",
                        tag=f"apad{i}")
        nc.gpsimd.memset(ap[:, :, 0:35], 0.0)
        gaps = ap[:, :, 67 : 67 + 34 * HW].rearrange(
            "p b (h w) -> p b h w", w=PADW
        )[:, :, :, 0:2]
        nc.gpsimd.memset(gaps, 0.0)
        nc.gpsimd.memset(ap[:, :, 35 + 34 * HW - 2 : IMG_PAD], 0.0)
        apad[i] = ap

    # ---------------- local BN stats ----------------
    stats_rec = []
    for b in range(CB):
        xb = xin[b]
        rec = stat_p.tile([128, 2 * NLOC, 6], F32, name=f"rec{b}", tag=f"rec{b}")
        for i in range(NLOC):
            for h in range(2):
                nc.vector.bn_stats(
                    out=rec[:, 2 * i + h, :],
                    in_=xb[:, i, 16 * h : 16 * (h + 1), :].rearrange(
                        "p h w -> p (h w)"
                    ),
                )
        stats_rec.append(rec)

    # pack [mean_b, meansq_b] per ci-block into AllReduce payload
    arbuf = stat_p.tile([128, 2 * CB], F32, name="arbuf")
    tmp1 = stat_p.tile([128, 1], F32, name="tmp1")
    for b in range(CB):
        mv = stat_p.tile([128, 2], F32, name=f"mv{b}", tag=f"mv{b}")
        nc.vector.bn_aggr(out=mv[:], in_=stats_rec[b][:])
        nc.vector.tensor_copy(out=arbuf[:, 2 * b : 2 * b + 1], in_=mv[:, 0:1])
        nc.vector.tensor_mul(tmp1[:], mv[:, 0:1], mv[:, 0:1])
        nc.vector.tensor_add(arbuf[:, 2 * b + 1 : 2 * b + 2], mv[:, 1:2], tmp1[:])

    nc.gpsimd.dma_start(out=cc_in[:, :], in_=arbuf[:])
    # AllGather + local 8-way add instead of AllReduce: the gather is a
    # single ring pass; the local reduction is 3 tree adds on DVE.
    nc.gpsimd.collective_compute(
        "AllGather",
        mybir.AluOpType.bypass,
        replica_groups=[list(range(N_CORES))],
        ins=[cc_in.ap().opt()],
        outs=[cc_out.ap().opt()],
    )

    # readback all 8 ranks' partials: the transfer is descriptor-bound (16B
    # granules), so split it across the three DGE-capable queues
    gsall = stat_p.tile([128, N_CORES, 2 * CB], F32, name="gsall")
    rb_split = [(nc.sync, 0, 3), (nc.scalar, 3, 6), (nc.gpsimd, 6, 8)]
    for eng, r0, r1 in rb_split:
        eng.dma_start(
            out=gsall[:, r0:r1, :],
            in_=cc_out[r0:r1, :, :].rearrange("k p s -> p k s"),
        )

    # tiny readback of rank 0's contiguous 2KB block: unblocks the PE
    # warm-up matmuls (issued after the gsall slices so it never delays them)
    junk_f32 = stat_p.tile([128, 2 * CB], F32, name="junk_f32")
    nc.gpsimd.dma_start(out=junk_f32[:], in_=cc_out[0, :, :])
    gs = stat_p.tile([128, 2 * CB], F32, name="gs")
    nc.vector.tensor_reduce(
        out=gs[:],
        in_=gsall[:].rearrange("p k s -> p s k"),
        axis=mybir.AxisListType.X,
        op=mybir.AluOpType.add,
    )
    smean = gs[:].rearrange("p (b s) -> p b s", s=2)[:, :, 0]  # [128, CB]
    smsq = gs[:].rearrange("p (b s) -> p b s", s=2)[:, :, 1]

    # per-channel scale/shift computed as wide [128, CB] ops.
    # Since std > 0:  sign((x-mean)*gamma/std + beta)
    #              == sign(gamma*x + (beta*std - mean*gamma))
    # so scale = gamma (known before the AllReduce!) and
    # shift = beta*std - mean*gamma  (no reciprocal needed).
    inv = 1.0 / N_CORES
    # PE warm-up trigger: gpsimd (not ACT/DVE, which are on the critical
    # stat-math path) casts the AllReduce readback into the warm-up rhs.
    junk = stat_p.tile([128, 4], BF16, name="junk")
    nc.gpsimd.tensor_copy(out=junk[:], in_=junk_f32[:])
    # msqr = (smean*inv)^2 without materializing the mean
    msqr = stat_p.tile([128, CB], F32, name="msqr")
    nc.vector.scalar_tensor_tensor(
        out=msqr[:],
        in0=smean,
        scalar=inv * inv,
        in1=smean,
        op0=mybir.AluOpType.mult,
        op1=mybir.AluOpType.mult,
    )
    var_t = stat_p.tile([128, CB], F32, name="var_t")
    # var = (smsq * inv) - mean^2
    nc.vector.scalar_tensor_tensor(
        out=var_t[:],
        in0=smsq,
        scalar=inv,
        in1=msqr[:],
        op0=mybir.AluOpType.mult,
        op1=mybir.AluOpType.subtract,
    )
    # neg_mg = -(mean)*gamma, computed on DVE in parallel with var
    neg_mg = stat_p.tile([128, CB], F32, name="neg_mg")
    nc.vector.scalar_tensor_tensor(
        out=neg_mg[:],
        in0=smean,
        scalar=-inv,
        in1=gam[:],
        op0=mybir.AluOpType.mult,
        op1=mybir.AluOpType.mult,
    )
    # sqrt on ScalarE (table preloaded); the two shift ops run on DVE so the
    # Scalar queue goes straight from Sqrt to the binarize Signs
    std_t = stat_p.tile([128, CB], F32, name="std_t")
    nc.scalar.activation(
        out=std_t[:],
        in_=var_t[:],
        func=mybir.ActivationFunctionType.Sqrt,
        bias=eps_t[:],
        scale=1.0,
    )
    sh_t = stat_p.tile([128, CB], F32, name="sh_t")
    nc.vector.tensor_mul(sh_t[:], std_t[:], bet[:])
    nc.vector.tensor_add(sh_t[:], sh_t[:], neg_mg[:])
    scale_t = [gam[:, b : b + 1] for b in range(CB)]
    shift_t = [sh_t[:, b : b + 1] for b in range(CB)]

    # warm-up matmuls (results discarded) — bridge the stat-math + binarize
    # window so the PE clock stays un-throttled into the conv burst. The
    # first one is gated on the AllGather readback via `junk`.
    pwj = ps_tr.tile([128, 4], F32, name="pw2", tag="ptr", bufs=2)
    nc.tensor.matmul(pwj[:], ident[:], junk[:], start=True, stop=True)
    for k in range(20):
        pw = ps_tr.tile([128, 256], F32, name="pw3", tag="ptr", bufs=2)
        nc.tensor.matmul(pw[:], ident[:], jw[:], start=True, stop=True)

    # ---------------- binarize into padded layout (fp8, DoubleRow pairs) ----
    # Row-halves so the first conv chunk (rows 0..13) can start as soon as
    # the top halves of both ci-blocks are written.
    for i in range(NLOC):
        for h in range(2):
            for b in range(CB):
                interior = apad[i][:, b, 35 : 35 + 34 * HW].rearrange(
                    "p (h w) -> p h w", w=PADW
                )[:, 16 * h : 16 * (h + 1), 0:HW]
                nc.scalar.activation(
                    out=interior,
                    in_=xin[b][:, i, 16 * h : 16 * (h + 1), :],
                    func=mybir.ActivationFunctionType.Sign,
                    scale=scale_t[b],
                    bias=shift_t[b],
                )

    # ---------------- conv: 9 shifted DoubleRow matmuls, PSUM accumulate ----
    for i in range(NLOC):
        psum = {}
        for o in range(OB):
            for ci, (r0, r1) in enumerate(CHUNKS):
                psum[(o, ci)] = ps_acc.tile(
                    [128, (r1 - r0) * PADW], F32, name=f"acc{o}_{ci}",
                    tag=f"acc{o}_{ci}", bufs=1,
                )
        for t, (dy, dx) in enumerate(TAPS):
            toff = dy * PADW + dx
            first = t == 0
            last = t == len(TAPS) - 1
            for o in range(OB):
                lhsT = wT[:, t * OB + o, :, :]
                for ci, (r0, r1) in enumerate(CHUNKS):
                    ncols = (r1 - r0) * PADW
                    off = r0 * PADW + toff
                    nc.tensor.matmul(
                        psum[(o, ci)][:],
                        lhsT,
                        apad[i][:, :, off : off + ncols],
                        start=first,
                        stop=last,
                        perf_mode=mybir.MatmulPerfMode.DoubleRow,
                    )
        last_img = i == NLOC - 1
        for o in range(OB):
            osb = out_p.tile([128, HW, HW], F32, name=f"osb{o}", tag=f"osb{o}",
                             bufs=2)
            for ci, (r0, r1) in enumerate(CHUNKS):
                src = psum[(o, ci)][:].rearrange("p (r c) -> p r c", c=PADW)[
                    :, :, 0:HW
                ]
                # For the final image, split the drain across two compute
                # engines and two DMA-issue queues so the two o-halves'
                # tails run in parallel instead of serializing on DVE+Sync.
                if last_img and o == 1:
                    nc.scalar.activation(
                        out=osb[:, r0:r1, :], in_=src,
                        func=mybir.ActivationFunctionType.Identity,
                    )
                    store_eng = nc.gpsimd
                else:
                    nc.vector.tensor_copy(out=osb[:, r0:r1, :], in_=src)
                    store_eng = nc.sync
                # per-chunk output DMA so the store of the final chunks
                # overlaps the remaining matmuls instead of tailing the kernel
                store_eng.dma_start(
                    out=y_d[i, 128 * o : 128 * (o + 1), r0:r1, :],
                    in_=osb[:, r0:r1, :],
                )


def _dedup_ldweights(nc):
    """Remove InstLdweights that reload the identical weights the PE already
    holds (the 3 row-chunk matmuls per (tap, o-block) share one lhsT).
    Restricted to perf-mode (conv) loads; any semaphore waits/updates on a
    removed load migrate to the next PE instruction (its matmul), which
    preserves ordering since both sit on the same engine queue."""
    removed = []
    for blk in nc.main_func.blocks:
        new_insts = []
        last_sig = None
        pending_sync = []  # SyncInfo objects from removed loads
        for inst in blk.instructions:
            if isinstance(inst, mybir.InstLdweights):
                ap = inst.ins[0]
                pm = inst.perf_mode
                sig = (str(pm), ap.memref, ap.offset, str(ap.ap))
                if pm is not None and sig == last_sig:
                    if inst.sync_info is not None:
                        pending_sync.append(inst.sync_info)
                    removed.append(inst.name)
                    continue
                last_sig = sig
                new_insts.append(inst)
                continue
            if getattr(inst, "engine", None) == mybir.EngineType.PE:
                if pending_sync:
                    si = inst.sync_info
                    if si is None:
                        si = mybir.SyncInfo(on_wait=[], on_update=[])
                        inst.sync_info = si
                    for ps in pending_sync:
                        si.on_wait.extend(ps.on_wait)
                        si.on_update.extend(ps.on_update)
                    pending_sync = []
                if isinstance(inst, mybir.InstMatmult):
                    if getattr(inst, "is_transpose", False):
                        last_sig = None  # transpose streams via the array
                elif not isinstance(
                    inst, (mybir.InstEventSemaphore, mybir.InstDrain)
                ):
                    last_sig = None  # unknown PE inst may clobber weights
            new_insts.append(inst)
        assert not pending_sync
        try:
            blk.instructions[:] = new_insts
        except TypeError:
            blk.instructions = new_insts
    for name in removed:
        nc.inst_map.pop(name, None)
    return len(removed)


_CACHE: dict = {}


def _build():
    if "nc" in _CACHE:
        return _CACHE["nc"]
    nc = bacc.Bacc(
        "TRN2", target_bir_lowering=False, debug=False, num_devices=N_CORES
    )
    x_d = nc.dram_tensor("x", [NLOC, C, HW, HW], F32, kind="ExternalInput")
    g_d = nc.dram_tensor("gamma", [C, 1], F32, kind="ExternalInput")
    be_d = nc.dram_tensor("beta", [C, 1], F32, kind="ExternalInput")
    w_d = nc.dram_tensor(
        "w", [128, len(TAPS) * OB, CB, 128], FP8, kind="ExternalInput"
    )
    y_d = nc.dram_tensor("y", [NLOC, C, HW, HW], F32, kind="ExternalOutput")
    cc_in = nc.dram_tensor("cc_in", [128, 2 * CB], F32)
    cc_out = nc.dram_tensor(
        "cc_out", [N_CORES, 128, 2 * CB], F32, addr_space="Shared"
    )

    from contextlib import ExitStack

    with tile.TileContext(nc) as tc, ExitStack() as ctx:
        _build_body(ctx, nc, tc, x_d, g_d, be_d, w_d, y_d, cc_in, cc_out)
    _dedup_ldweights(nc)
    nc.compile()
    _CACHE["nc"] = nc
    return nc


def _pack_weights(W: np.ndarray) -> np.ndarray:
    """sign(W) packed to wT[ci_local, tap*OB+o_blk, ci_blk, o_local] fp8."""
    ws = np.sign(W.astype(np.float32))
    # [o_blk, o_l, ci_blk, ci_l, dy, dx] -> [ci_l, dy, dx, o_blk, ci_blk, o_l]
    ws = ws.reshape(OB, 128, CB, 128, 3, 3).transpose(3, 4, 5, 0, 2, 1)
    ws = np.ascontiguousarray(ws.reshape(128, len(TAPS) * OB, CB, 128))
    return ws.astype(mybir.dt.np(FP8))


def kernel(x, gamma, beta, W):
    x = np.ascontiguousarray(np.asarray(x, dtype=np.float32))
    gamma = np.ascontiguousarray(np.asarray(gamma, dtype=np.float32)).reshape(C, 1)
    beta = np.ascontiguousarray(np.asarray(beta, dtype=np.float32)).reshape(C, 1)
    wT = _pack_weights(np.asarray(W, dtype=np.float32))
    nc = _build()
    in_maps = [
        {
            "x": x[NLOC * k : NLOC * (k + 1)],
            "gamma": gamma,
            "beta": beta,
            "w": wT,
        }
        for k in range(N_CORES)
    ]
    res = run_bass_kernel_spmd(nc, in_maps, core_ids=list(range(N_CORES)))
    return np.concatenate(
        [res.results[k]["y"] for k in range(N_CORES)], axis=0
    )


# revision 29
# speedup vs baseline: 1.0001x; 1.0001x over previous
"""BinaryConv (BN + sign-binarize + 3x3 binary conv) on 8 Trainium2 NeuronCores.

Strategy (data-parallel over batch, per sharding hint):
  - Each of the 8 cores gets 4 of the 32 images; weights/gamma/beta replicated.
  - Weights are host-prepped (sign + transpose to DoubleRow [ci][tap*o][half][o]
    fp8 layout) since they are tiny replicated constants; the device loads the
    packed 590KB tensor directly.
  - A tiny dummy AllGather is triggered first so the one-time collective
    rendezvous/arming cost overlaps the x load + local-stats phase instead of
    sitting on the critical path of the real stats AllGather.
  - Per-core BN partial stats (mean, mean-square per channel) via bn_stats,
    AllGathered across cores (tiny 2KB payload) and tree-reduced locally on
    DVE: sync-BN exact.
  - Binarize via ScalarE Sign(gamma*x + (beta*std - mean*gamma)) — equivalent
    to the reference BN+sign since std>0 — into a zero-padded per-image
    layout (34-wide rows, both ci-halves stacked) in fp8 e4m3.
  - 3x3 conv = 9 shifted DoubleRow fp8 matmuls (contraction 256 in one pass)
    accumulated in PSUM. +/-1 inputs with fp32 PSUM accumulation are exact
    (integer sums), so the conv matches the fp32 reference bit-for-bit.
  - A long chain of PE warm-up matmuls spans the stats/collective window so
    the HAM clock gate is fully ramped when the conv burst starts.
"""

import numpy as np

import concourse.tile as tile
from concourse import bacc, mybir
from concourse.bass_utils import run_bass_kernel_spmd
from concourse.masks import make_identity

F32 = mybir.dt.float32
BF16 = mybir.dt.bfloat16
FP8 = mybir.dt.float8e4

N_CORES = 8
N = 32            # full batch
NLOC = N // N_CORES  # images per core
C = 256           # channels (in == out)
HW = 32           # spatial
CB = C // 128     # ci partition blocks
OB = C // 128     # o partition blocks
EPS = 1e-5

PADW = HW + 2     # padded row width
IMG_PAD = 1160    # per-image padded buffer (>= 34*34 + 2 margin, mult of 8)
# output row-chunks (r0, r1): each chunk's matmul free dim = (r1-r0)*34 <= 512
CHUNKS = [(0, 11), (11, 22), (22, 32)]
TAPS = [(dy, dx) for dy in range(3) for dx in range(3)]
N_PRE_WARM = 52   # PE ramp matmuls spanning the stats/collective window


def _build_body(ctx, nc, tc, x_d, g_d, be_d, w_d, y_d, cc_in, cc_out):
    # ---------------- pools ----------------
    const = ctx.enter_context(tc.tile_pool(name="const", bufs=1))
    xin_p = ctx.enter_context(tc.tile_pool(name="xin", bufs=1))
    wpool = ctx.enter_context(tc.tile_pool(name="wpool", bufs=1))
    apool = ctx.enter_context(tc.tile_pool(name="apool", bufs=1))
    stat_p = ctx.enter_context(tc.tile_pool(name="stat", bufs=1))
    out_p = ctx.enter_context(tc.tile_pool(name="outp", bufs=1))
    ps_tr = ctx.enter_context(tc.tile_pool(name="pstr", bufs=2, space="PSUM"))
    ps_acc = ctx.enter_context(tc.tile_pool(name="psacc", bufs=1, space="PSUM"))

    # ---------------- load x (stats-critical) ----------------
    # Alternate the two HWDGE issue engines (Sync, Scalar) to halve the
    # descriptor-generation serialization on the input path.
    xin = []
    for b in range(CB):
        xb = xin_p.tile([128, NLOC, HW, HW], F32, name=f"xin{b}", tag=f"xin{b}")
        for i in range(NLOC):
            eng = nc.sync if (b * NLOC + i) % 2 == 0 else nc.scalar
            eng.dma_start(
                out=xb[:, i], in_=x_d[i, 128 * b : 128 * (b + 1), :, :]
            )
        xin.append(xb)

    # prepacked binarized weights, DoubleRow layout (host-prepared):
    # wT[ci_local, tap*OB+o, ci_half, o_local]  (fp8)
    wT = wpool.tile([128, len(TAPS) * OB, CB, 128], FP8, name="wT")
    nc.scalar.dma_start(out=wT[:], in_=w_d[:, :, :, :])

    # gamma/beta: tiny, issue right after the x/w descriptors
    gam = stat_p.tile([128, CB], F32, name="gam")
    bet = stat_p.tile([128, CB], F32, name="bet")
    for b in range(CB):
        nc.sync.dma_start(out=gam[:, b : b + 1], in_=g_d[128 * b : 128 * (b + 1), :])
        nc.sync.dma_start(out=bet[:, b : b + 1], in_=be_d[128 * b : 128 * (b + 1), :])

    # ---------------- scalar act-table preloads (Sign + Sqrt) ---------------
    eps_t = const.tile([128, 1], F32, name="eps_t")
    nc.vector.memset(eps_t[:], EPS)
    scrap = const.tile([128, 1], F32, name="scrap")
    nc.scalar.activation(
        out=scrap[:], in_=eps_t[:], func=mybir.ActivationFunctionType.Sign
    )
    nc.scalar.activation(
        out=scrap[:],
        in_=eps_t[:],
        func=mybir.ActivationFunctionType.Sqrt,
        bias=eps_t[:],
        scale=1.0,
    )

    # ---------------- PE warm-up chain through the stats window -------------
    # fp32 matmuls (4 cyc/row) give long-running filler with few
    # instructions: ~52 x ~850ns spans ~8..52us, keeping the PE clock
    # ramped until just before the conv burst.
    ident = const.tile([128, 128], BF16, name="ident")
    make_identity(nc, ident[:])
    jw = const.tile([128, 256], BF16, name="jw")
    nc.vector.memset(jw[:], 1.0)
    jwf = const.tile([128, 512], F32, name="jwf")
    nc.vector.memset(jwf[:], 1.0)
    for k in range(N_PRE_WARM):
        pw = ps_tr.tile([128, 512], F32, name="pw", tag="ptr", bufs=2)
        nc.tensor.matmul(pw[:], jwf[:, 0:128], jwf[:], start=True, stop=True)

    # ---------------- zero only the padding of the activation buffers ------
    # (interior is fully overwritten by binarize; tiny strided memsets keep
    # both DVE and the collective-trigger path free)
    apad = [None] * NLOC
    for i in range(NLOC):
        ap = apool.tile([128, CB, IMG_PAD], FP8, name=f"apad{i}",
                        tag=f"apad{i}")
        nc.gpsimd.memset(ap[:, :, 0:35], 0.0)
        gaps = ap[:, :, 67 : 67 + 34 * HW].rearrange(
            "p b (h w) -> p b h w", w=PADW
        )[:, :, :, 0:2]
        nc.gpsimd.memset(gaps, 0.0)
        nc.gpsimd.memset(ap[:, :, 35 + 34 * HW - 2 : IMG_PAD], 0.0)
        apad[i] = ap

    # ---------------- local BN stats ----------------
    stats_rec = []
    for b in range(CB):
        xb = xin[b]
        rec = stat_p.tile([128, 2 * NLOC, 6], F32, name=f"rec{b}", tag=f"rec{b}")
        for i in range(NLOC):
            for h in range(2):
                nc.vector.bn_stats(
                    out=rec[:, 2 * i + h, :],
                    in_=xb[:, i, 16 * h : 16 * (h + 1), :].rearrange(
                        "p h w -> p (h w)"
                    ),
                )
        stats_rec.append(rec)

    # pack [mean_b, meansq_b] per ci-block into AllReduce payload
    arbuf = stat_p.tile([128, 2 * CB], F32, name="arbuf")
    tmp1 = stat_p.tile([128, 1], F32, name="tmp1")
    for b in range(CB):
        mv = stat_p.tile([128, 2], F32, name=f"mv{b}", tag=f"mv{b}")
        nc.vector.bn_aggr(out=mv[:], in_=stats_rec[b][:])
        nc.vector.tensor_copy(out=arbuf[:, 2 * b : 2 * b + 1], in_=mv[:, 0:1])
        nc.vector.tensor_mul(tmp1[:], mv[:, 0:1], mv[:, 0:1])
        nc.vector.tensor_add(arbuf[:, 2 * b + 1 : 2 * b + 2], mv[:, 1:2], tmp1[:])

    nc.gpsimd.dma_start(out=cc_in[:, :], in_=arbuf[:])
    # AllGather + local 8-way add instead of AllReduce: the gather is a
    # single ring pass; the local reduction is 3 tree adds on DVE.
    nc.gpsimd.collective_compute(
        "AllGather",
        mybir.AluOpType.bypass,
        replica_groups=[list(range(N_CORES))],
        ins=[cc_in.ap().opt()],
        outs=[cc_out.ap().opt()],
    )

    # readback all 8 ranks' partials: the transfer is descriptor-bound (16B
    # granules), so split it across the three DGE-capable queues
    gsall = stat_p.tile([128, N_CORES, 2 * CB], F32, name="gsall")
    rb_split = [(nc.sync, 0, 3), (nc.scalar, 3, 6), (nc.gpsimd, 6, 8)]
    for eng, r0, r1 in rb_split:
        eng.dma_start(
            out=gsall[:, r0:r1, :],
            in_=cc_out[r0:r1, :, :].rearrange("k p s -> p k s"),
        )

    # tiny readback of rank 0's contiguous 2KB block: unblocks the PE
    # warm-up matmuls (issued after the gsall slices so it never delays the
    # gpsimd slice the first tree add waits on)
    junk_f32 = stat_p.tile([128, 2 * CB], F32, name="junk_f32")
    nc.gpsimd.dma_start(out=junk_f32[:], in_=cc_out[0, :, :])
    gs4 = stat_p.tile([128, 4, 2 * CB], F32, name="gs4")
    nc.vector.tensor_add(gs4[:], gsall[:, 0:4, :], gsall[:, 4:8, :])
    gs2 = stat_p.tile([128, 2, 2 * CB], F32, name="gs2")
    nc.vector.tensor_add(gs2[:], gs4[:, 0:2, :], gs4[:, 2:4, :])
    gs = stat_p.tile([128, 2 * CB], F32, name="gs")
    nc.vector.tensor_add(gs[:], gs2[:, 0, :], gs2[:, 1, :])
    smean = gs[:].rearrange("p (b s) -> p b s", s=2)[:, :, 0]  # [128, CB]
    smsq = gs[:].rearrange("p (b s) -> p b s", s=2)[:, :, 1]

    # per-channel scale/shift computed as wide [128, CB] ops.
    # Since std > 0:  sign((x-mean)*gamma/std + beta)
    #              == sign(gamma*x + (beta*std - mean*gamma))
    # so scale = gamma (known before the AllReduce!) and
    # shift = beta*std - mean*gamma  (no reciprocal needed).
    inv = 1.0 / N_CORES
    # PE warm-up trigger: gpsimd (not ACT/DVE, which are on the critical
    # stat-math path) casts the AllReduce readback into the warm-up rhs.
    junk = stat_p.tile([128, 4], BF16, name="junk")
    nc.gpsimd.tensor_copy(out=junk[:], in_=junk_f32[:])
    # msqr = (smean*inv)^2 without materializing the mean
    msqr = stat_p.tile([128, CB], F32, name="msqr")
    nc.vector.scalar_tensor_tensor(
        out=msqr[:],
        in0=smean,
        scalar=inv * inv,
        in1=smean,
        op0=mybir.AluOpType.mult,
        op1=mybir.AluOpType.mult,
    )
    var_t = stat_p.tile([128, CB], F32, name="var_t")
    # var = (smsq * inv) - mean^2
    nc.vector.scalar_tensor_tensor(
        out=var_t[:],
        in0=smsq,
        scalar=inv,
        in1=msqr[:],
        op0=mybir.AluOpType.mult,
        op1=mybir.AluOpType.subtract,
    )
    # neg_mg = -(mean)*gamma, computed on DVE in parallel with var
    neg_mg = stat_p.tile([128, CB], F32, name="neg_mg")
    nc.vector.scalar_tensor_tensor(
        out=neg_mg[:],
        in0=smean,
        scalar=-inv,
        in1=gam[:],
        op0=mybir.AluOpType.mult,
        op1=mybir.AluOpType.mult,
    )
    # sqrt on ScalarE (table preloaded); the two shift ops run on DVE so the
    # Scalar queue goes straight from Sqrt to the binarize Signs
    std_t = stat_p.tile([128, CB], F32, name="std_t")
    nc.scalar.activation(
        out=std_t[:],
        in_=var_t[:],
        func=mybir.ActivationFunctionType.Sqrt,
        bias=eps_t[:],
        scale=1.0,
    )
    sh_t = stat_p.tile([128, CB], F32, name="sh_t")
    nc.vector.tensor_mul(sh_t[:], std_t[:], bet[:])
    nc.vector.tensor_add(sh_t[:], sh_t[:], neg_mg[:])
    scale_t = [gam[:, b : b + 1] for b in range(CB)]
    shift_t = [sh_t[:, b : b + 1] for b in range(CB)]

    # warm-up matmuls (results discarded) — bridge the stat-math + binarize
    # window so the PE clock stays un-throttled into the conv burst. The
    # first one is gated on the AllGather readback via `junk`.
    pwj = ps_tr.tile([128, 4], F32, name="pw2", tag="ptr", bufs=2)
    nc.tensor.matmul(pwj[:], ident[:], junk[:], start=True, stop=True)
    for k in range(20):
        pw = ps_tr.tile([128, 256], F32, name="pw3", tag="ptr", bufs=2)
        nc.tensor.matmul(pw[:], ident[:], jw[:], start=True, stop=True)

    # ---------------- binarize into padded layout (fp8, DoubleRow pairs) ----
    # Row-halves so the first conv chunk (rows 0..13) can start as soon as
    # the top halves of both ci-blocks are written.
    for i in range(NLOC):
        for h in range(2):
            for b in range(CB):
                interior = apad[i][:, b, 35 : 35 + 34 * HW].rearrange(
                    "p (h w) -> p h w", w=PADW
                )[:, 16 * h : 16 * (h + 1), 0:HW]
                nc.scalar.activation(
                    out=interior,
                    in_=xin[b][:, i, 16 * h : 16 * (h + 1), :],
                    func=mybir.ActivationFunctionType.Sign,
                    scale=scale_t[b],
                    bias=shift_t[b],
                )

    # ---------------- conv: 9 shifted DoubleRow matmuls, PSUM accumulate ----
    for i in range(NLOC):
        psum = {}
        for o in range(OB):
            for ci, (r0, r1) in enumerate(CHUNKS):
                psum[(o, ci)] = ps_acc.tile(
                    [128, (r1 - r0) * PADW], F32, name=f"acc{o}_{ci}",
                    tag=f"acc{o}_{ci}", bufs=1,
                )
        for t, (dy, dx) in enumerate(TAPS):
            toff = dy * PADW + dx
            first = t == 0
            last = t == len(TAPS) - 1
            for o in range(OB):
                lhsT = wT[:, t * OB + o, :, :]
                for ci, (r0, r1) in enumerate(CHUNKS):
                    ncols = (r1 - r0) * PADW
                    off = r0 * PADW + toff
                    nc.tensor.matmul(
                        psum[(o, ci)][:],
                        lhsT,
                        apad[i][:, :, off : off + ncols],
                        start=first,
                        stop=last,
                        perf_mode=mybir.MatmulPerfMode.DoubleRow,
                    )
        last_img = i == NLOC - 1
        for o in range(OB):
            osb = out_p.tile([128, HW, HW], F32, name=f"osb{o}", tag=f"osb{o}",
                             bufs=2)
            for ci, (r0, r1) in enumerate(CHUNKS):
                src = psum[(o, ci)][:].rearrange("p (r c) -> p r c", c=PADW)[
                    :, :, 0:HW
                ]
                # For the final image, split the drain across two compute
                # engines and two DMA-issue queues so the two o-halves'
                # tails run in parallel instead of serializing on DVE+Sync.
                if last_img and o == 1:
                    nc.scalar.activation(
                        out=osb[:, r0:r1, :], in_=src,
                        func=mybir.ActivationFunctionType.Identity,
                    )
                    store_eng = nc.gpsimd
                else:
                    nc.vector.tensor_copy(out=osb[:, r0:r1, :], in_=src)
                    store_eng = nc.sync
                # per-chunk output DMA so the store of the final chunks
                # overlaps the remaining matmuls instead of tailing the kernel
                store_eng.dma_start(
                    out=y_d[i, 128 * o : 128 * (o + 1), r0:r1, :],
                    in_=osb[:, r0:r1, :],
                )


def _dedup_ldweights(nc):
    """Remove InstLdweights that reload the identical weights the PE already
    holds (the 3 row-chunk matmuls per (tap, o-block) share one lhsT).
    Restricted to perf-mode (conv) loads; any semaphore waits/updates on a
    removed load migrate to the next PE instruction (its matmul), which
    preserves ordering since both sit on the same engine queue."""
    removed = []
    for blk in nc.main_func.blocks:
        new_insts = []
        last_sig = None
        pending_sync = []  # SyncInfo objects from removed loads
        for inst in blk.instructions:
            if isinstance(inst, mybir.InstLdweights):
                ap = inst.ins[0]
                pm = inst.perf_mode
                sig = (str(pm), ap.memref, ap.offset, str(ap.ap))
                if pm is not None and sig == last_sig:
                    if inst.sync_info is not None:
                        pending_sync.append(inst.sync_info)
                    removed.append(inst.name)
                    continue
                last_sig = sig
                new_insts.append(inst)
                continue
            if getattr(inst, "engine", None) == mybir.EngineType.PE:
                if pending_sync:
                    si = inst.sync_info
                    if si is None:
                        si = mybir.SyncInfo(on_wait=[], on_update=[])
                        inst.sync_info = si
                    for ps in pending_sync:
                        si.on_wait.extend(ps.on_wait)
                        si.on_update.extend(ps.on_update)
                    pending_sync = []
                if isinstance(inst, mybir.InstMatmult):
                    if getattr(inst, "is_transpose", False):
                        last_sig = None  # transpose streams via the array
                elif not isinstance(
                    inst, (mybir.InstEventSemaphore, mybir.InstDrain)
                ):
                    last_sig = None  # unknown PE inst may clobber weights
            new_insts.append(inst)
        assert not pending_sync
        try:
            blk.instructions[:] = new_insts
        except TypeError:
            blk.instructions = new_insts
    for name in removed:
        nc.inst_map.pop(name, None)
    return len(removed)


_CACHE: dict = {}


def _build():
    if "nc" in _CACHE:
        return _CACHE["nc"]
    nc = bacc.Bacc(
        "TRN2", target_bir_lowering=False, debug=False, num_devices=N_CORES
    )
    x_d = nc.dram_tensor("x", [NLOC, C, HW, HW], F32, kind="ExternalInput")
    g_d = nc.dram_tensor("gamma", [C, 1], F32, kind="ExternalInput")
    be_d = nc.dram_tensor("beta", [C, 1], F32, kind="ExternalInput")
    w_d = nc.dram_tensor(
        "w", [128, len(TAPS) * OB, CB, 128], FP8, kind="ExternalInput"
    )
    y_d = nc.dram_tensor("y", [NLOC, C, HW, HW], F32, kind="ExternalOutput")
    cc_in = nc.dram_tensor("cc_in", [128, 2 * CB], F32)
    cc_out = nc.dram_tensor(
        "cc_out", [N_CORES, 128, 2 * CB], F32, addr_space="Shared"
    )

    from contextlib import ExitStack

    with tile.TileContext(nc) as tc, ExitStack() as ctx:
        _build_body(ctx, nc, tc, x_d, g_d, be_d, w_d, y_d, cc_in, cc_out)
    _dedup_ldweights(nc)
    nc.compile()
    _CACHE["nc"] = nc
    return nc


def _pack_weights(W: np.ndarray) -> np.ndarray:
    """sign(W) packed to wT[ci_local, tap*OB+o_blk, ci_blk, o_local] fp8."""
    ws = np.sign(W.astype(np.float32))
    # [o_blk, o_l, ci_blk, ci_l, dy, dx] -> [ci_l, dy, dx, o_blk, ci_blk, o_l]
    ws = ws.reshape(OB, 128, CB, 128, 3, 3).transpose(3, 4, 5, 0, 2, 1)
    ws = np.ascontiguousarray(ws.reshape(128, len(TAPS) * OB, CB, 128))
    return ws.astype(mybir.dt.np(FP8))


def kernel(x, gamma, beta, W):
    x = np.ascontiguousarray(np.asarray(x, dtype=np.float32))
    gamma = np.ascontiguousarray(np.asarray(gamma, dtype=np.float32)).reshape(C, 1)
    beta = np.ascontiguousarray(np.asarray(beta, dtype=np.float32)).reshape(C, 1)
    wT = _pack_weights(np.asarray(W, dtype=np.float32))
    nc = _build()
    in_maps = [
        {
            "x": x[NLOC * k : NLOC * (k + 1)],
            "gamma": gamma,
            "beta": beta,
            "w": wT,
        }
        for k in range(N_CORES)
    ]
    res = run_bass_kernel_spmd(nc, in_maps, core_ids=list(range(N_CORES)))
    return np.concatenate(
        [res.results[k]["y"] for k in range(N_CORES)], axis=0
    )


# revision 30
# speedup vs baseline: 1.0108x; 1.0107x over previous
"""BinaryConv (BN + sign-binarize + 3x3 binary conv) on 8 Trainium2 NeuronCores.

Strategy (data-parallel over batch, per sharding hint):
  - Each of the 8 cores gets 4 of the 32 images; weights/gamma/beta replicated.
  - Weights are host-prepped (sign + transpose to DoubleRow [ci][tap*o][half][o]
    fp8 layout) since they are tiny replicated constants; the device loads the
    packed 590KB tensor directly.
  - Per-core BN partial stats (mean, mean-square per channel) via bn_stats,
    AllGathered across cores (tiny 2KB payload) and tree-reduced locally on
    DVE: sync-BN exact.
  - Binarize via ScalarE Sign(gamma*x + (beta*std - mean*gamma)) — equivalent
    to the reference BN+sign since std>0 — into a zero-padded per-image
    layout (34-wide rows, both ci-halves stacked) in fp8 e4m3.
  - 3x3 conv = 9 shifted DoubleRow fp8 matmuls (contraction 256 in one pass)
    accumulated in PSUM. +/-1 inputs with fp32 PSUM accumulation are exact
    (integer sums), so the conv matches the fp32 reference bit-for-bit.
  - A long chain of PE warm-up matmuls spans the stats/collective window so
    the HAM clock gate is fully ramped when the conv burst starts.
"""

import numpy as np

import concourse.tile as tile
from concourse import bacc, mybir
from concourse.bass_utils import run_bass_kernel_spmd
from concourse.masks import make_identity

F32 = mybir.dt.float32
BF16 = mybir.dt.bfloat16
FP8 = mybir.dt.float8e4

N_CORES = 8
N = 32            # full batch
NLOC = N // N_CORES  # images per core
C = 256           # channels (in == out)
HW = 32           # spatial
CB = C // 128     # ci partition blocks
OB = C // 128     # o partition blocks
EPS = 1e-5

PADW = HW + 2     # padded row width
IMG_PAD = 1160    # per-image padded buffer (>= 34*34 + 2 margin, mult of 8)
# output row-chunks (r0, r1): each chunk's matmul free dim = (r1-r0)*34 <= 512
CHUNKS = [(0, 11), (11, 22), (22, 32)]
TAPS = [(dy, dx) for dy in range(3) for dx in range(3)]
N_PRE_WARM = 52   # PE ramp matmuls spanning the stats/collective window


def _build_body(ctx, nc, tc, x_d, g_d, be_d, w_d, y_d, cc_in, cc_out):
    # ---------------- pools ----------------
    const = ctx.enter_context(tc.tile_pool(name="const", bufs=1))
    xin_p = ctx.enter_context(tc.tile_pool(name="xin", bufs=1))
    wpool = ctx.enter_context(tc.tile_pool(name="wpool", bufs=1))
    apool = ctx.enter_context(tc.tile_pool(name="apool", bufs=1))
    stat_p = ctx.enter_context(tc.tile_pool(name="stat", bufs=1))
    out_p = ctx.enter_context(tc.tile_pool(name="outp", bufs=1))
    ps_tr = ctx.enter_context(tc.tile_pool(name="pstr", bufs=2, space="PSUM"))
    ps_acc = ctx.enter_context(tc.tile_pool(name="psacc", bufs=1, space="PSUM"))

    # ---------------- load x (stats-critical) ----------------
    # Alternate the two HWDGE issue engines (Sync, Scalar) to halve the
    # descriptor-generation serialization on the input path.
    xin = []
    for b in range(CB):
        xb = xin_p.tile([128, NLOC, HW, HW], F32, name=f"xin{b}", tag=f"xin{b}")
        for i in range(NLOC):
            eng = nc.sync if (b * NLOC + i) % 2 == 0 else nc.scalar
            eng.dma_start(
                out=xb[:, i], in_=x_d[i, 128 * b : 128 * (b + 1), :, :]
            )
        xin.append(xb)

    # prepacked binarized weights, DoubleRow layout (host-prepared):
    # wT[ci_local, tap*OB+o, ci_half, o_local]  (fp8)
    wT = wpool.tile([128, len(TAPS) * OB, CB, 128], FP8, name="wT")
    nc.scalar.dma_start(out=wT[:], in_=w_d[:, :, :, :])

    # gamma/beta: tiny, issue right after the x/w descriptors
    gam = stat_p.tile([128, CB], F32, name="gam")
    bet = stat_p.tile([128, CB], F32, name="bet")
    for b in range(CB):
        nc.sync.dma_start(out=gam[:, b : b + 1], in_=g_d[128 * b : 128 * (b + 1), :])
        nc.sync.dma_start(out=bet[:, b : b + 1], in_=be_d[128 * b : 128 * (b + 1), :])

    # ---------------- scalar act-table preloads (Sign + Sqrt) ---------------
    eps_t = const.tile([128, 1], F32, name="eps_t")
    nc.vector.memset(eps_t[:], EPS)
    scrap = const.tile([128, 1], F32, name="scrap")
    nc.scalar.activation(
        out=scrap[:], in_=eps_t[:], func=mybir.ActivationFunctionType.Sign
    )
    nc.scalar.activation(
        out=scrap[:],
        in_=eps_t[:],
        func=mybir.ActivationFunctionType.Sqrt,
        bias=eps_t[:],
        scale=1.0,
    )

    # ---------------- PE warm-up chain through the stats window -------------
    # fp32 matmuls (4 cyc/row) give long-running filler with few
    # instructions: ~52 x ~850ns spans ~8..52us, keeping the PE clock
    # ramped until just before the conv burst.
    ident = const.tile([128, 128], BF16, name="ident")
    make_identity(nc, ident[:])
    jw = const.tile([128, 256], BF16, name="jw")
    nc.vector.memset(jw[:], 1.0)
    jwf = const.tile([128, 512], F32, name="jwf")
    nc.vector.memset(jwf[:], 1.0)
    for k in range(N_PRE_WARM):
        pw = ps_tr.tile([128, 512], F32, name="pw", tag="ptr", bufs=2)
        nc.tensor.matmul(pw[:], jwf[:, 0:128], jwf[:], start=True, stop=True)

    # ---------------- zero only the padding of the activation buffers ------
    # (interior is fully overwritten by binarize; tiny strided memsets keep
    # both DVE and the collective-trigger path free)
    apad = [None] * NLOC
    for i in range(NLOC):
        ap = apool.tile([128, CB, IMG_PAD], FP8, name=f"apad{i}",
                        tag=f"apad{i}")
        nc.gpsimd.memset(ap[:, :, 0:35], 0.0)
        gaps = ap[:, :, 67 : 67 + 34 * HW].rearrange(
            "p b (h w) -> p b h w", w=PADW
        )[:, :, :, 0:2]
        nc.gpsimd.memset(gaps, 0.0)
        nc.gpsimd.memset(ap[:, :, 35 + 34 * HW - 2 : IMG_PAD], 0.0)
        apad[i] = ap

    # ---------------- local BN stats ----------------
    stats_rec = []
    for b in range(CB):
        xb = xin[b]
        rec = stat_p.tile([128, 2 * NLOC, 6], F32, name=f"rec{b}", tag=f"rec{b}")
        for i in range(NLOC):
            for h in range(2):
                nc.vector.bn_stats(
                    out=rec[:, 2 * i + h, :],
                    in_=xb[:, i, 16 * h : 16 * (h + 1), :].rearrange(
                        "p h w -> p (h w)"
                    ),
                )
        stats_rec.append(rec)

    # pack [mean_b, meansq_b] per ci-block into AllReduce payload
    arbuf = stat_p.tile([128, 2 * CB], F32, name="arbuf")
    tmp1 = stat_p.tile([128, 1], F32, name="tmp1")
    for b in range(CB):
        mv = stat_p.tile([128, 2], F32, name=f"mv{b}", tag=f"mv{b}")
        nc.vector.bn_aggr(out=mv[:], in_=stats_rec[b][:])
        nc.vector.tensor_copy(out=arbuf[:, 2 * b : 2 * b + 1], in_=mv[:, 0:1])
        nc.vector.tensor_mul(tmp1[:], mv[:, 0:1], mv[:, 0:1])
        nc.vector.tensor_add(arbuf[:, 2 * b + 1 : 2 * b + 2], mv[:, 1:2], tmp1[:])

    nc.gpsimd.dma_start(out=cc_in[:, :], in_=arbuf[:])
    # AllGather + local 8-way add instead of AllReduce: the gather is a
    # single ring pass; the local reduction is 3 tree adds on DVE.
    nc.gpsimd.collective_compute(
        "AllGather",
        mybir.AluOpType.bypass,
        replica_groups=[list(range(N_CORES))],
        ins=[cc_in.ap().opt()],
        outs=[cc_out.ap().opt()],
    )

    # readback all 8 ranks' partials: the transfer is descriptor-bound (16B
    # granules), so split it across the three DGE-capable queues
    gsall = stat_p.tile([128, N_CORES, 2 * CB], F32, name="gsall")
    rb_split = [(nc.sync, 0, 3), (nc.scalar, 3, 6), (nc.gpsimd, 6, 8)]
    for eng, r0, r1 in rb_split:
        eng.dma_start(
            out=gsall[:, r0:r1, :],
            in_=cc_out[r0:r1, :, :].rearrange("k p s -> p k s"),
        )

    # tiny readback of rank 0's contiguous 2KB block: unblocks the PE
    # warm-up matmuls (issued after the gsall slices so it never delays the
    # gpsimd slice the first tree add waits on)
    junk_f32 = stat_p.tile([128, 2 * CB], F32, name="junk_f32")
    nc.gpsimd.dma_start(out=junk_f32[:], in_=cc_out[0, :, :])
    gs4 = stat_p.tile([128, 4, 2 * CB], F32, name="gs4")
    nc.vector.tensor_add(gs4[:], gsall[:, 0:4, :], gsall[:, 4:8, :])
    gs2 = stat_p.tile([128, 2, 2 * CB], F32, name="gs2")
    nc.vector.tensor_add(gs2[:], gs4[:, 0:2, :], gs4[:, 2:4, :])
    gs = stat_p.tile([128, 2 * CB], F32, name="gs")
    nc.vector.tensor_add(gs[:], gs2[:, 0, :], gs2[:, 1, :])
    smean = gs[:].rearrange("p (b s) -> p b s", s=2)[:, :, 0]  # [128, CB]
    smsq = gs[:].rearrange("p (b s) -> p b s", s=2)[:, :, 1]

    # per-channel scale/shift computed as wide [128, CB] ops.
    # Since std > 0:  sign((x-mean)*gamma/std + beta)
    #              == sign(gamma*x + (beta*std - mean*gamma))
    # so scale = gamma (known before the AllReduce!) and
    # shift = beta*std - mean*gamma  (no reciprocal needed).
    inv = 1.0 / N_CORES
    # PE warm-up trigger: gpsimd (not ACT/DVE, which are on the critical
    # stat-math path) casts the AllReduce readback into the warm-up rhs.
    junk = stat_p.tile([128, 4], BF16, name="junk")
    nc.gpsimd.tensor_copy(out=junk[:], in_=junk_f32[:])
    # msqr = (smean*inv)^2 without materializing the mean
    msqr = stat_p.tile([128, CB], F32, name="msqr")
    nc.vector.scalar_tensor_tensor(
        out=msqr[:],
        in0=smean,
        scalar=inv * inv,
        in1=smean,
        op0=mybir.AluOpType.mult,
        op1=mybir.AluOpType.mult,
    )
    var_t = stat_p.tile([128, CB], F32, name="var_t")
    # var = (smsq * inv) - mean^2
    nc.vector.scalar_tensor_tensor(
        out=var_t[:],
        in0=smsq,
        scalar=inv,
        in1=msqr[:],
        op0=mybir.AluOpType.mult,
        op1=mybir.AluOpType.subtract,
    )
    # neg_mg = -(mean)*gamma, computed on DVE in parallel with var
    neg_mg = stat_p.tile([128, CB], F32, name="neg_mg")
    nc.vector.scalar_tensor_tensor(
        out=neg_mg[:],
        in0=smean,
        scalar=-inv,
        in1=gam[:],
        op0=mybir.AluOpType.mult,
        op1=mybir.AluOpType.mult,
    )
    # sqrt on ScalarE (table preloaded); the two shift ops run on DVE so the
    # Scalar queue goes straight from Sqrt to the binarize Signs
    std_t = stat_p.tile([128, CB], F32, name="std_t")
    nc.scalar.activation(
        out=std_t[:],
        in_=var_t[:],
        func=mybir.ActivationFunctionType.Sqrt,
        bias=eps_t[:],
        scale=1.0,
    )
    sh_t = stat_p.tile([128, CB], F32, name="sh_t")
    nc.vector.tensor_mul(sh_t[:], std_t[:], bet[:])
    nc.vector.tensor_add(sh_t[:], sh_t[:], neg_mg[:])
    scale_t = [gam[:, b : b + 1] for b in range(CB)]
    shift_t = [sh_t[:, b : b + 1] for b in range(CB)]

    # warm-up matmuls (results discarded) — bridge the stat-math + binarize
    # window so the PE clock stays un-throttled into the conv burst. The
    # first one is gated on the AllGather readback via `junk`.
    pwj = ps_tr.tile([128, 4], F32, name="pw2", tag="ptr", bufs=2)
    nc.tensor.matmul(pwj[:], ident[:], junk[:], start=True, stop=True)
    for k in range(20):
        pw = ps_tr.tile([128, 256], F32, name="pw3", tag="ptr", bufs=2)
        nc.tensor.matmul(pw[:], ident[:], jw[:], start=True, stop=True)

    # ---------------- binarize into padded layout (fp8, DoubleRow pairs) ----
    # Row-halves so the first conv chunk (rows 0..13) can start as soon as
    # the top halves of both ci-blocks are written.
    for i in range(NLOC):
        for h in range(2):
            for b in range(CB):
                interior = apad[i][:, b, 35 : 35 + 34 * HW].rearrange(
                    "p (h w) -> p h w", w=PADW
                )[:, 16 * h : 16 * (h + 1), 0:HW]
                nc.scalar.activation(
                    out=interior,
                    in_=xin[b][:, i, 16 * h : 16 * (h + 1), :],
                    func=mybir.ActivationFunctionType.Sign,
                    scale=scale_t[b],
                    bias=shift_t[b],
                )

    # ---------------- conv: 9 shifted DoubleRow matmuls, PSUM accumulate ----
    for i in range(NLOC):
        psum = {}
        for o in range(OB):
            for ci, (r0, r1) in enumerate(CHUNKS):
                psum[(o, ci)] = ps_acc.tile(
                    [128, (r1 - r0) * PADW], F32, name=f"acc{o}_{ci}",
                    tag=f"acc{o}_{ci}", bufs=1,
                )
        for t, (dy, dx) in enumerate(TAPS):
            toff = dy * PADW + dx
            first = t == 0
            last = t == len(TAPS) - 1
            for o in range(OB):
                lhsT = wT[:, t * OB + o, :, :]
                for ci, (r0, r1) in enumerate(CHUNKS):
                    ncols = (r1 - r0) * PADW
                    off = r0 * PADW + toff
                    nc.tensor.matmul(
                        psum[(o, ci)][:],
                        lhsT,
                        apad[i][:, :, off : off + ncols],
                        start=first,
                        stop=last,
                        perf_mode=mybir.MatmulPerfMode.DoubleRow,
                    )
        last_img = i == NLOC - 1
        for o in range(OB):
            osb = out_p.tile([128, HW, HW], F32, name=f"osb{o}", tag=f"osb{o}",
                             bufs=2)
            for ci, (r0, r1) in enumerate(CHUNKS):
                src = psum[(o, ci)][:].rearrange("p (r c) -> p r c", c=PADW)[
                    :, :, 0:HW
                ]
                # For the final image, split the drain across two compute
                # engines and two DMA-issue queues so the two o-halves'
                # tails run in parallel instead of serializing on DVE+Sync.
                if last_img and o == 1:
                    nc.scalar.activation(
                        out=osb[:, r0:r1, :], in_=src,
                        func=mybir.ActivationFunctionType.Identity,
                    )
                    store_eng = nc.gpsimd
                else:
                    nc.vector.tensor_copy(out=osb[:, r0:r1, :], in_=src)
                    store_eng = nc.sync
                # per-chunk output DMA so the store of the final chunks
                # overlaps the remaining matmuls instead of tailing the kernel
                store_eng.dma_start(
                    out=y_d[i, 128 * o : 128 * (o + 1), r0:r1, :],
                    in_=osb[:, r0:r1, :],
                )


def _dedup_ldweights(nc):
    """Remove InstLdweights that reload the identical weights the PE already
    holds (the 3 row-chunk matmuls per (tap, o-block) share one lhsT).
    Restricted to perf-mode (conv) loads; any semaphore waits/updates on a
    removed load migrate to the next PE instruction (its matmul), which
    preserves ordering since both sit on the same engine queue."""
    removed = []
    for blk in nc.main_func.blocks:
        new_insts = []
        last_sig = None
        pending_sync = []  # SyncInfo objects from removed loads
        for inst in blk.instructions:
            if isinstance(inst, mybir.InstLdweights):
                ap = inst.ins[0]
                pm = inst.perf_mode
                sig = (str(pm), ap.memref, ap.offset, str(ap.ap))
                if pm is not None and sig == last_sig:
                    if inst.sync_info is not None:
                        pending_sync.append(inst.sync_info)
                    removed.append(inst.name)
                    continue
                last_sig = sig
                new_insts.append(inst)
                continue
            if getattr(inst, "engine", None) == mybir.EngineType.PE:
                if pending_sync:
                    si = inst.sync_info
                    if si is None:
                        si = mybir.SyncInfo(on_wait=[], on_update=[])
                        inst.sync_info = si
                    for ps in pending_sync:
                        si.on_wait.extend(ps.on_wait)
                        si.on_update.extend(ps.on_update)
                    pending_sync = []
                if isinstance(inst, mybir.InstMatmult):
                    if getattr(inst, "is_transpose", False):
                        last_sig = None  # transpose streams via the array
                elif not isinstance(
                    inst, (mybir.InstEventSemaphore, mybir.InstDrain)
                ):
                    last_sig = None  # unknown PE inst may clobber weights
            new_insts.append(inst)
        assert not pending_sync
        try:
            blk.instructions[:] = new_insts
        except TypeError:
            blk.instructions = new_insts
    for name in removed:
        nc.inst_map.pop(name, None)
    return len(removed)


_CACHE: dict = {}


def _build():
    if "nc" in _CACHE:
        return _CACHE["nc"]
    nc = bacc.Bacc(
        "TRN2", target_bir_lowering=False, debug=False, num_devices=N_CORES
    )
    x_d = nc.dram_tensor("x", [NLOC, C, HW, HW], F32, kind="ExternalInput")
    g_d = nc.dram_tensor("gamma", [C, 1], F32, kind="ExternalInput")
    be_d = nc.dram_tensor("beta", [C, 1], F32, kind="ExternalInput")
    w_d = nc.dram_tensor(
        "w", [128, len(TAPS) * OB, CB, 128], FP8, kind="ExternalInput"
    )
    y_d = nc.dram_tensor("y", [NLOC, C, HW, HW], F32, kind="ExternalOutput")
    cc_in = nc.dram_tensor("cc_in", [128, 2 * CB], F32)
    cc_out = nc.dram_tensor(
        "cc_out", [N_CORES, 128, 2 * CB], F32, addr_space="Shared"
    )

    from contextlib import ExitStack

    with tile.TileContext(nc) as tc, ExitStack() as ctx:
        _build_body(ctx, nc, tc, x_d, g_d, be_d, w_d, y_d, cc_in, cc_out)
    _dedup_ldweights(nc)
    nc.compile()
    _CACHE["nc"] = nc
    return nc


def _pack_weights(W: np.ndarray) -> np.ndarray:
    """sign(W) packed to wT[ci_local, tap*OB+o_blk, ci_blk, o_local] fp8."""
    ws = np.sign(W.astype(np.float32))
    # [o_blk, o_l, ci_blk, ci_l, dy, dx] -> [ci_l, dy, dx, o_blk, ci_blk, o_l]
    ws = ws.reshape(OB, 128, CB, 128, 3, 3).transpose(3, 4, 5, 0, 2, 1)
    ws = np.ascontiguousarray(ws.reshape(128, len(TAPS) * OB, CB, 128))
    return ws.astype(mybir.dt.np(FP8))


def kernel(x, gamma, beta, W):
    x = np.ascontiguousarray(np.asarray(x, dtype=np.float32))
    gamma = np.ascontiguousarray(np.asarray(gamma, dtype=np.float32)).reshape(C, 1)
    beta = np.ascontiguousarray(np.asarray(beta, dtype=np.float32)).reshape(C, 1)
    wT = _pack_weights(np.asarray(W, dtype=np.float32))
    nc = _build()
    in_maps = [
        {
            "x": x[NLOC * k : NLOC * (k + 1)],
            "gamma": gamma,
            "beta": beta,
            "w": wT,
        }
        for k in range(N_CORES)
    ]
    res = run_bass_kernel_spmd(nc, in_maps, core_ids=list(range(N_CORES)))
    return np.concatenate(
        [res.results[k]["y"] for k in range(N_CORES)], axis=0
    )
